# revision 49
# baseline (speedup 1.0000x reference)
"""Trainium2 Bass kernel for a BasicTransformerBlock (self-attn + cross-attn + GEGLU FF).

Sharding: 8 cores = 2 batches x 4 sequence chunks of 1024 rows. Each core
redundantly computes LN1 + K/V projections over its batch's full 4096 rows
(position-independent, so all cores run an identical SPMD program) and
produces its own 1024-row slice of the output. No collectives.

Precision: fp32 residual stream; fp8e4m3 weights/activations with DoubleRow
matmuls for every projection and the attention score / probability-x-V
products. On real TRN2 silicon DoubleRow does NOT speed up the moving-dim
stream (~0.55ns/output-col regardless of dtype at the observed clock) —
its win is doubling contract per pass, which halves the pass count of every
projection (contract 256+ per DR matmul). Weights are pre-scaled x16 on the
host so their 0.02-sigma values sit in fp8's normal range; the 1/16 (or
1/256 for Q.K) is folded into activation scales, the softmax ones-column
(set to 16), and the residual-add epilogues. Softmax runs without
max-subtraction (scores are provably small at this scale) with the scale
folded into exp; the denominator comes free from a ones-column in V.

Engine budget: LN rstd is computed entirely on DVE (batched stats + a
Newton rsqrt from a 1/x seed), so outside the FF gelu the ACT engine runs a
single activation table — no 1.3us ACT_TABLE_LOAD thrash. Softmax exp is
split 5:3 between ACT (table exp) and DVE ((1+z/32)^32 custom op); all
PSUM->SBUF copies round-robin ACT/DVE (GpSimd cannot read PSUM and its
tensor ops are ~6x slower than DVE, so it only does memsets). Known floor:
the score stream (262k PE columns) and the LDWEIGHTS-bound PV phase
(~0.47ns per stationary column, reloaded per 128-q-block) dominate; per-core
clock state varies run-to-run by ~20% (normalize by the EXP-op average when
comparing traces).
"""

import numpy as np
import ml_dtypes

DIM = 320
HEADS = 8
DH = 40
CTX = 768
IFF = 1280  # GEGLU inner width; proj1 width = 2*IFF
EPS = 1e-5
SCALE = DH ** -0.5
NCORES = 8
MCTX = 77
VS = 336  # V row stride (8*41 = 328 padded to %16 for DoubleRow pair stride)
WS = 16.0  # host-side weight scale (folded back out on-device)

BF16 = ml_dtypes.bfloat16
E4M3 = ml_dtypes.float8_e4m3


def _register_exp_op():
    """Custom DVE op: out = (in0*s0 + s1)^32 — used as exp(z) ~ (1+z/32)^32
    to offload part of the softmax exp from ACT to the vector engine."""
    import concourse.dve_ops as dve_ops
    for o in dve_ops.OPS:
        if o.name == "EXP_POLY32_ANT":
            return o
    from concourse.dve_spec import Spec, Src0, C0, C1, sq
    spec = Spec(
        body=sq(sq(sq(sq(sq(Src0 * C0 + C1))))),
        reference=lambda in0, in1, s0, s1, imm2:
            ((in0.astype(np.float32) * s0 + s1) ** 32).astype(np.float32))
    op = dve_ops.DveOp("EXP_POLY32_ANT", spec, subdim=False,
                       uops_sha={"v3": "eafb894a1d5c531b"})
    dve_ops.OPS.append(op)
    dve_ops._SUB_OPCODE_FOR_NAME[op.name] = \
        dve_ops._CUSTOM_DVE_ROW_BASE + len(dve_ops.OPS) - 1
    dve_ops.CUSTOM_DVE_SPECS[op.name] = op.spec
    return op


def build_nc(S, R, flags=()):
    """Build + compile the per-core Bass program.

    flags: subset of {"ln1_w","ln1_b","ln2_w","ln2_b","ln3_w","ln3_b",
    "a1_bo","a2_bo","ff_b2"} that are non-trivial and must be applied.
    """
    import concourse.bass as bass
    import concourse.tile as tile
    from concourse import bacc, mybir
    from concourse.masks import make_identity

    f32 = mybir.dt.float32
    bf = mybir.dt.bfloat16
    f8 = mybir.dt.float8e4
    AF = mybir.ActivationFunctionType
    OP = mybir.AluOpType
    PM = mybir.MatmulPerfMode
    flags = set(flags)

    KB = S // 128     # key blocks (self-attn)
    KBP = KB // 2     # key block pairs (DoubleRow PV)
    QT = R // 128     # q row-tiles
    QHS = R // 512    # q 512-row groups
    ISC = SCALE / (WS * WS)  # exp scale with both Q,K weight scales folded

    nc = bacc.Bacc("TRN2", target_bir_lowering=False, debug=False)

    def din(name, shape, dt=f8):
        return nc.dram_tensor(name, shape, dt, kind="ExternalInput").ap()

    xfull_d = din("xfull", [S, DIM], bf)
    xq_d = din("xq", [R, DIM], f32)
    xqb_d = din("xqb", [R, DIM], bf)
    xqs_d = din("xqs", [R, 2], f32)
    xfs_d = din("xfs", [S, 2], f32)
    ctxT_d = din("ctxT", [CTX, MCTX])
    # weights: [in(padded), out] fp8e4m3, host-prescaled x16
    w_d = {}
    for nm, shape in [
        ("a1_Wq", [512, 512]), ("a1_Wk", [512, 512]), ("a1_Wv", [512, DIM]),
        ("a1_Wo", [DIM, DIM]), ("a2_Wq", [512, 512]), ("a2_Wk", [CTX, 512]),
        ("a2_Wv", [CTX, DIM]), ("a2_Wo", [DIM, DIM]),
        ("ff_W1", [512, 2 * IFF]), ("ff_W2", [IFF, DIM]),
    ]:
        w_d[nm] = din(nm, shape)
    b1_d = din("ff_b1", [2 * IFF], f32)
    vec_d = {nm: din(nm, [DIM], f32) for nm in sorted(flags)}
    out_d = nc.dram_tensor("out", [R, DIM], f32, kind="ExternalOutput").ap()

    NCH = {512: 4, CTX: 6, IFF: 10, DIM: 3}  # input-dim -> #128-chunks

    with tile.TileContext(nc) as tc:
        import contextlib
        with contextlib.ExitStack() as est:
            persist = est.enter_context(tc.tile_pool(name="persist", bufs=1))
            work = est.enter_context(tc.tile_pool(name="work", bufs=4))
            expp = est.enter_context(tc.tile_pool(name="expp", bufs=6))
            # One PSUM pool: tag "sc" = 2 x [128,1024]f32 (4 banks), tag
            # "acc" = 4 x [128,512]f32 (4 banks). All other PSUM tiles
            # allocate from these tags so phases can pipeline.
            psum = est.enter_context(tc.tile_pool(name="psum", bufs=2,
                                                  space="PSUM"))

            def ps_sc(shape, dt=f32, name="sc"):
                return psum.tile(shape, dt, tag="sc", bufs=2, name=name)

            def ps_acc(shape, dt=f32, name="accp"):
                return psum.tile(shape, dt, tag="acc", bufs=4, name=name)

            ident = persist.tile([128, 128], bf, name="ident")
            make_identity(nc, ident)
            # ---- persistent activations (feature-major, 4 chunk-slots for
            # DoubleRow pairing; chunk2[64:] and chunk3 zeroed once)
            h1T = persist.tile([128, 4, S], f8, name="h1T")
            actT = persist.tile([128, 4, R], f8, name="actT")
            Kf = persist.tile([128, 4, S], f8, name="Kf")
            Qf = persist.tile([128, 4, R], f8, name="Qf")
            Vr = persist.tile([128, KB, VS], f8, name="Vr")
            K2f = persist.tile([128, 4, 80], f8, name="K2f")
            Q2f = persist.tile([128, 4, R], f8, name="Q2f")
            V2r = persist.tile([128, VS], f8, name="V2r")
            resid = persist.tile([128, QT, DIM], f32, name="resid")
            Uff = persist.tile([128, IFF // 128, R], f8, name="Uff")

            # actT pads first (small, gate the first transposes + Qf);
            # h1T pads after (only needed once K-proj starts)
            nc.gpsimd.memset(actT[64:128, 2, :], 0.0)
            nc.gpsimd.memset(actT[:, 3, :], 0.0)
            nc.gpsimd.memset(h1T[64:128, 2, :], 0.0)
            nc.gpsimd.memset(h1T[:, 3, :], 0.0)

            # ---- weights into SBUF, [in, out] layout chunked on partitions.
            wsb = {}

            def load_w(names):
                for nm in names:
                    ind, width = w_d[nm].shape
                    nch = NCH[ind]
                    t = persist.tile([128, nch, width], f8, name=f"w_{nm}",
                                     uniquify=True)
                    for c in range(nch):
                        kw = min(128, ind - c * 128)
                        nc.sync.dma_start(out=t[:kw, c, :],
                                          in_=w_d[nm][c * 128:c * 128 + kw, :])
                    wsb[nm] = t

            xqs = persist.tile([128, QT, 2], f32, name="xqs")
            xfs = persist.tile([128, KB, 2], f32, name="xfs")
            nc.sync.dma_start(out=xqs,
                              in_=xqs_d.rearrange("(t p) s -> p t s", p=128))
            nc.sync.dma_start(out=xfs,
                              in_=xfs_d.rearrange("(t p) s -> p t s", p=128))
            xqb = persist.tile([128, QT, DIM], bf, name="xqb")
            for t in range(QT):
                nc.sync.dma_start(out=xqb[:, t, :],
                                  in_=xqb_d[t * 128:(t + 1) * 128, :])
            load_w(["a1_Wq", "a1_Wk", "a1_Wv"])
            for t in range(QT):
                nc.sync.dma_start(out=resid[:, t, :],
                                  in_=xq_d[t * 128:(t + 1) * 128, :])

            bcast = {}
            for nm in sorted(flags):
                t = persist.tile([128, DIM], f32, name=f"bc_{nm}")
                src = vec_d[nm]
                bc_ap = bass.AP(tensor=src.tensor, offset=src.offset,
                                ap=[[0, 128]] + [list(p) for p in src.ap])
                nc.gpsimd.dma_start(out=t, in_=bc_ap)
                bcast[nm] = t

            # round-robin engine pickers for PSUM->SBUF copies
            _cp = [0]

            def copy_eng():
                _cp[0] += 1
                return "act" if _cp[0] % 2 == 0 else "dve"

            def ps_copy(out, in_, eng=None, scale=None, bias=None):
                eng = eng or copy_eng()
                if eng == "act":
                    nc.scalar.activation(out, in_, AF.Identity,
                                         bias=bias if bias is not None else 0.0,
                                         scale=scale if scale is not None else 1.0)
                else:
                    assert bias is None and scale is None
                    nc.vector.tensor_copy(out=out, in_=in_)

            def ln_stats_alloc(n):
                return work.tile([128, n, 2], f32, tag="mv", bufs=3,
                                 name="mvall")

            def ln_stat1(mvall, t, src_ap):
                stats = work.tile([128, 6], f32, tag="bnst", name="stats")
                nc.vector.bn_stats(stats, src_ap)
                nc.vector.bn_aggr(mvall[:, t, :], stats)

            def ln_finish(mvall, n, src_fn, wkey, bkey, consume_fn):
                """rstd = rsqrt(var) via seed y0=(1+1/v)/2 + 3 Newton steps
                (DVE only — no ACT table functions), then per-tile normalize
                handed to consume_fn(t, h)."""
                var_ap = mvall[:, :, 1]
                rstd = work.tile([128, n], f32, tag="rstd", name="rstd")
                tmp = work.tile([128, n], f32, tag="rtmp", name="rtmp")
                nc.vector.reciprocal(rstd, var_ap)
                nc.vector.tensor_scalar(out=rstd, in0=rstd, scalar1=0.5,
                                        scalar2=0.5, op0=OP.mult, op1=OP.add)
                for _ in range(2):
                    nc.vector.tensor_mul(out=tmp, in0=rstd, in1=rstd)
                    nc.vector.tensor_mul(out=tmp, in0=tmp, in1=var_ap)
                    nc.vector.tensor_scalar(out=tmp, in0=tmp, scalar1=-0.5,
                                            scalar2=1.5, op0=OP.mult, op1=OP.add)
                    nc.vector.tensor_mul(out=rstd, in0=rstd, in1=tmp)
                for t in range(n):
                    h = work.tile([128, DIM], bf, tag="h", bufs=6, name="h")
                    nc.vector.tensor_scalar(
                        out=h, in0=src_fn(t), scalar1=mvall[:, t, 0:1],
                        scalar2=rstd[:, t:t + 1], op0=OP.subtract, op1=OP.mult)
                    if wkey in flags:
                        nc.vector.tensor_mul(out=h, in0=h, in1=bcast[wkey])
                    if bkey in flags:
                        nc.vector.tensor_add(out=h, in0=h, in1=bcast[bkey])
                    consume_fn(t, h)

            def ln_single(src_ap, wkey, bkey, consume_fn):
                """Latency-optimized per-tile LN: full stats->rsqrt->normalize
                chain for ONE 128-row tile, so its transpose can issue without
                waiting for a whole phase's stats batch."""
                mv = ln_stats_alloc(1)
                ln_stat1(mv, 0, src_ap)
                ln_finish(mv, 1, lambda _t: src_ap, wkey, bkey,
                          lambda _t, h: consume_fn(h))

            def ln_norm_host(src_ap, st, t, wkey, bkey):
                """LN1 normalize using host-precomputed (mean, rstd)."""
                h = work.tile([128, DIM], bf, tag="h", bufs=6, name="h")
                nc.vector.tensor_scalar(
                    out=h, in0=src_ap, scalar1=st[:, t, 0:1],
                    scalar2=st[:, t, 1:2], op0=OP.subtract, op1=OP.mult)
                if wkey in flags:
                    nc.vector.tensor_mul(out=h, in0=h, in1=bcast[wkey])
                if bkey in flags:
                    nc.vector.tensor_add(out=h, in0=h, in1=bcast[bkey])
                return h

            def ln_batch(n, src_fn, wkey, bkey, consume_fn):
                mvall = ln_stats_alloc(n)
                for t in range(n):
                    ln_stat1(mvall, t, src_fn(t))
                ln_finish(mvall, n, src_fn, wkey, bkey, consume_fn)

            def transpose_into(dstT, src_bf, col0, ps_fn=None):
                for c in range(3):
                    kw = 128 if c < 2 else 64
                    pt = (ps_fn or ps_sc)([128, 128], bf, name="tr_ps")
                    nc.tensor.transpose(pt[:kw, :], src_bf[:, c * 128:c * 128 + kw],
                                        ident)
                    ps_copy(dstT[:kw, c, col0:col0 + 128], pt[:kw, :])

            def proj_fm(dst, wt, srcT, n_lo, n_hi, npair, eng=None):
                """Feature-major projection, DoubleRow over chunk pairs.
                dst[:, g, n0:n0+nw] = (wt[:,:,128g:128(g+1)]).T @ srcT[:,:,n]"""
                ng = wt.shape[2] // 128
                for g in range(ng):
                    for n0 in range(n_lo, n_hi, 512):
                        nw = min(512, n_hi - n0)
                        ps = ps_acc([128, 512], name="proj_ps")
                        for p in range(npair):
                            nc.tensor.matmul(
                                ps[:, :nw],
                                lhsT=wt[:, 2 * p:2 * p + 2, 128 * g:128 * g + 128],
                                rhs=srcT[:, 2 * p:2 * p + 2, n0:n0 + nw],
                                start=(p == 0), stop=(p == npair - 1),
                                perf_mode=PM.DoubleRow)
                        ps_copy(dst[:, g, n0:n0 + nw], ps[:, :nw], eng=eng)

            exp_op = _register_exp_op()

            def load_late_weights():
                load_w(["a1_Wo", "a2_Wq", "a2_Wk", "a2_Wv", "a2_Wo", "ff_W1",
                        "ff_W2"])
                b1 = persist.tile([128, (2 * IFF) // 128], f32, name="b1t")
                nc.sync.dma_start(out=b1, in_=b1_d.rearrange("(c p) -> p c", p=128))
                ctxm = persist.tile([128, 6, 80], f8, name="ctxT_sb")
                nc.gpsimd.memset(ctxm, 0.0)
                for c in range(6):
                    nc.sync.dma_start(out=ctxm[:, c, :MCTX],
                                      in_=ctxT_d[c * 128:(c + 1) * 128, :])
                return b1, ctxm

            def cross_kv():
                # K2: [128,4,80] fp8 in qk8 layout; V2: [77, 328] + ones(16)
                for g in range(4):
                    ps = ps_sc([128, 128], name="k2_ps")
                    for p in range(3):
                        nc.tensor.matmul(
                            ps[:, :80],
                            lhsT=wsb["a2_Wk"][:, 2 * p:2 * p + 2,
                                              128 * g:128 * g + 128],
                            rhs=ctxT_sb[:, 2 * p:2 * p + 2, :],
                            start=(p == 0), stop=(p == 2),
                            perf_mode=PM.DoubleRow)
                    nc.vector.tensor_copy(out=K2f[:, g, :], in_=ps[:, :80])
                ps = ps_acc([128, 512], name="v2_ps")
                for p in range(3):
                    nc.tensor.matmul(
                        ps[:80, :DIM],
                        lhsT=ctxT_sb[:, 2 * p:2 * p + 2, :],
                        rhs=wsb["a2_Wv"][:, 2 * p:2 * p + 2, :],
                        start=(p == 0), stop=(p == 2),
                        perf_mode=PM.DoubleRow)
                nc.vector.tensor_copy(
                    out=V2r[:MCTX, 0:328].rearrange("p (h c) -> p h c", c=41)[:, :, 0:40],
                    in_=ps[:MCTX, :DIM].rearrange("p (h c) -> p h c", c=40))
                nc.vector.memset(
                    V2r[:MCTX, 0:328].rearrange("p (h c) -> p h c",
                                                c=41)[:, :, 40:41], WS)

            xf = persist.tile([128, KB, DIM], bf, name="xf")
            for t in range(KB):
                nc.sync.dma_start(out=xf[:, t, :],
                                  in_=xfull_d[t * 128:(t + 1) * 128, :])

            # ---- own rows: LN1 -> actT, Qf (stats host-precomputed)
            for t in range(QT):
                h = ln_norm_host(xqb[:, t, :], xqs, t, "ln1_w", "ln1_b")
                transpose_into(actT, h, t * 128)
            proj_fm(Qf, wsb["a1_Wq"], actT, 0, R, 2, eng="act")

            # ---- attn1 building blocks
            def qk8_lhs_rhs(h, kb, q0):
                g, m = divmod(h, 4)
                lhsT = Kf[32 * m:32 * m + 20, 2 * g:2 * g + 2,
                          kb * 128:(kb + 1) * 128]
                rhs = Qf[32 * m:32 * m + 20, 2 * g:2 * g + 2, q0:q0 + 512]
                return lhsT, rhs, (32 * m, 0)

            def attn1_scores_exp(q0, hp, kbp, ept):
                """Two kb blocks of scores+exp for heads (2hp, 2hp+1) into
                ept[:, i, :]. Exp split between ACT and DVE."""
                for i in range(2):
                    kb = 2 * kbp + i
                    sc = ps_sc([128, 1024], name="sc")
                    for j in range(2):
                        h = 2 * hp + j
                        lhsT, rhs, tp = qk8_lhs_rhs(h, kb, q0)
                        nc.tensor.matmul(sc[:, j * 512:(j + 1) * 512],
                                         lhsT=lhsT, rhs=rhs, start=True,
                                         stop=True, perf_mode=PM.DoubleRow,
                                         tile_position=tp)
                    if (2 * kbp + i + 3 * hp) % 8 < 3:
                        # exp(z) ~ (1+z/32)^32 on the vector engine
                        nc.vector._custom_dve(exp_op, out=ept[:, i, :], in0=sc,
                                              s0=ISC / 32.0, s1=1.0)
                    else:
                        nc.scalar.activation(ept[:, i, :], sc, AF.Exp, scale=ISC)

            def attn1_pv(acc, hp, kbp, ept):
                for j in range(2):
                    h = 2 * hp + j
                    for qs in range(4):
                        nc.tensor.matmul(
                            acc[qs][:, 41 * h:41 * h + 41],
                            lhsT=ept[:, :, j * 512 + qs * 128:
                                     j * 512 + (qs + 1) * 128],
                            rhs=Vr[:, 2 * kbp:2 * kbp + 2, 41 * h:41 * h + 41],
                            start=(kbp == 0), stop=(kbp == KBP - 1),
                            perf_mode=PM.DoubleRow, skip_group_check=True)

            # ---- LN1 + K/V production per 512-column block
            def kv_block(nb):
                for g in range(4):
                    ps = ps_acc([128, 512], name="kf_ps")
                    for p in range(2):
                        nc.tensor.matmul(
                            ps,
                            lhsT=wsb["a1_Wk"][:, 2 * p:2 * p + 2,
                                              128 * g:128 * g + 128],
                            rhs=h1T[:, 2 * p:2 * p + 2, nb * 512:(nb + 1) * 512],
                            start=(p == 0), stop=(p == 1),
                            perf_mode=PM.DoubleRow)
                    ps_copy(Kf[:, g, nb * 512:(nb + 1) * 512], ps)
                for tt in range(4):
                    t = nb * 4 + tt
                    ps = ps_acc([128, 512], name="v_ps")  # noqa: PLW2901
                    for p in range(2):
                        nc.tensor.matmul(
                            ps[:, :DIM],
                            lhsT=h1T[:, 2 * p:2 * p + 2, t * 128:(t + 1) * 128],
                            rhs=wsb["a1_Wv"][:, 2 * p:2 * p + 2, :],
                            start=(p == 0), stop=(p == 1),
                            perf_mode=PM.DoubleRow)
                    nc.vector.tensor_copy(
                        out=Vr[:, t, 0:328].rearrange("p (h c) -> p h c",
                                                      c=41)[:, :, 0:40],
                        in_=ps[:, :DIM].rearrange("p (h c) -> p h c", c=40))
                    if t % 8 == 7 or t == KB - 1:
                        lo = t - (t % 8)
                        nc.vector.memset(
                            Vr[:, lo:t + 1, 0:328].rearrange(
                                "p b (h c) -> p b h c", c=41)[:, :, :, 40], WS)

            def kv_consume(t, h):
                transpose_into(h1T, h, t * 128)
                if t % 4 == 3:
                    kv_block(t // 4)

            for t in range(KB):
                h = ln_norm_host(xf[:, t, :], xfs, t, "ln1_w", "ln1_b")
                kv_consume(t, h)

            # ---- shared attention epilogue: normalize, transpose, proj, add
            def finish_attn(qh, acc, wo, bo_key, after_qs=None):
                for qs in range(4):
                    rec = work.tile([128, HEADS], f32, tag="rec", name="rec")
                    nc.vector.reciprocal(
                        rec, acc[qs].rearrange("p (h c) -> p h c", c=41)[:, :, 40])
                    arm = work.tile([128, DIM], bf, tag="arm", name="arm")
                    rb = bass.AP(tensor=rec.tensor, offset=rec.offset,
                                 ap=[list(rec.ap[0]), [rec.ap[1][0], HEADS],
                                     [0, 40]])
                    nc.vector.tensor_mul(
                        out=arm.rearrange("p (h c) -> p h c", c=40),
                        in0=acc[qs].rearrange("p (h c) -> p h c", c=41)[:, :, 0:40],
                        in1=rb)
                    afm = work.tile([128, 3, 128], f8, tag="afm", name="afm")
                    for c in range(3):
                        kw = 128 if c < 2 else 64
                        pt = ps_acc([128, 128], bf, name="afm_tr")
                        nc.tensor.transpose(pt[:kw, :],
                                            arm[:, c * 128:c * 128 + kw], ident)
                        ps_copy(afm[:kw, c, :], pt[:kw, :])
                    po = ps_acc([128, DIM], name="po")
                    nc.tensor.matmul(po, lhsT=afm[:, 0:2, :], rhs=wo[:, 0:2, :],
                                     start=True, stop=False,
                                     perf_mode=PM.DoubleRow)
                    nc.tensor.matmul(po, lhsT=afm[:64, 2, :], rhs=wo[:64, 2, :],
                                     start=False, stop=True)
                    t = qh * 4 + qs
                    # resid += po/WS  (weight prescale folded out)
                    nc.vector.scalar_tensor_tensor(
                        out=resid[:, t, :], in0=po, scalar=1.0 / WS,
                        in1=resid[:, t, :], op0=OP.mult, op1=OP.add)
                    if bo_key in flags:
                        nc.vector.tensor_add(out=resid[:, t, :],
                                             in0=resid[:, t, :],
                                             in1=bcast[bo_key])
                    if after_qs is not None:
                        after_qs(qs)

            NMT = (2 * IFF) // 128  # 20
            # ================= per q-half: attn1 -> attn2 -> FF (pipelined)
            for qh in range(QHS):
                q0 = qh * 512
                # ---- self-attention (PV software-pipelined one pair back)
                acc = [ps_acc([128, HEADS * 41], name=f"acc{qs}")
                       for qs in range(4)]
                pending = []
                for hp in range(HEADS // 2):
                    for kbp in range(KBP):
                        ept = expp.tile([128, 2, 1024], f8, tag="ep", name="ep")
                        attn1_scores_exp(q0, hp, kbp, ept)
                        pending.append((hp, kbp, ept))
                        while len(pending) > 2:
                            attn1_pv(acc, *pending.pop(0))
                for phk in pending:
                    attn1_pv(acc, *phk)
                if qh == 0:
                    # cross-attn/FF weights + context K,V: DMA'd and computed
                    # here so they hide under attn1(qh0) instead of stalling
                    # the PE before it starts
                    b1t, ctxT_sb = load_late_weights()
                    cross_kv()
                finish_attn(qh, acc, wsb["a1_Wo"], "a1_bo",
                            after_qs=lambda qs: ln_single(
                                resid[:, qh * 4 + qs, :], "ln2_w", "ln2_b",
                                lambda h, _q=qs: transpose_into(
                                    actT, h, (qh * 4 + _q) * 128)))

                # ---- cross-attention for this q-half
                proj_fm(Q2f, wsb["a2_Wq"], actT, q0, q0 + 512, 2)
                acc = [ps_acc([128, HEADS * 41], name=f"acc2_{qs}")
                       for qs in range(4)]
                p2 = []
                for hp in range(HEADS // 2):
                    sc = ps_sc([128, 1024], name="sc2")
                    for j in range(2):
                        h = 2 * hp + j
                        g, m = h // 4, h % 4
                        nc.tensor.matmul(
                            sc[:80, j * 512:(j + 1) * 512],
                            lhsT=K2f[32 * m:32 * m + 20, 2 * g:2 * g + 2, :],
                            rhs=Q2f[32 * m:32 * m + 20, 2 * g:2 * g + 2,
                                    q0:q0 + 512],
                            start=True, stop=True, perf_mode=PM.DoubleRow,
                            tile_position=(32 * m, 0))
                    ep = expp.tile([128, 1024], f8, tag="ep2", bufs=4, name="ep2")
                    if hp % 2 == 1:
                        nc.vector._custom_dve(exp_op, out=ep[:MCTX, :],
                                              in0=sc[:MCTX, :],
                                              s0=ISC / 32.0, s1=1.0)
                    else:
                        nc.scalar.activation(ep[:MCTX, :], sc[:MCTX, :], AF.Exp,
                                             scale=ISC)
                    p2.append((hp, ep))
                for hp, ep in p2:
                    for j in range(2):
                        h = 2 * hp + j
                        for qs in range(4):
                            nc.tensor.matmul(
                                acc[qs][:, 41 * h:41 * h + 41],
                                lhsT=ep[:MCTX, j * 512 + qs * 128:
                                        j * 512 + (qs + 1) * 128],
                                rhs=V2r[:MCTX, 41 * h:41 * h + 41],
                                start=True, stop=True, skip_group_check=True)
                finish_attn(qh, acc, wsb["a2_Wo"], "a2_bo",
                            after_qs=lambda qs: ln_single(
                                resid[:, qh * 4 + qs, :], "ln3_w", "ln3_b",
                                lambda h, _q=qs: transpose_into(
                                    actT, h, (qh * 4 + _q) * 128)))

                # ---- GEGLU FF for this q-half
                _order = [m for pair in zip(range(NMT // 2), range(NMT // 2, NMT))
                          for m in pair]
                for mt in _order:
                    ps = ps_acc([128, 512], name="ff1_ps")
                    for p in range(2):
                        nc.tensor.matmul(
                            ps,
                            lhsT=wsb["ff_W1"][:, 2 * p:2 * p + 2,
                                              mt * 128:(mt + 1) * 128],
                            rhs=actT[:, 2 * p:2 * p + 2, q0:q0 + 512],
                            start=(p == 0), stop=(p == 1),
                            perf_mode=PM.DoubleRow)
                    if mt < NMT // 2:
                        nc.scalar.activation(Uff[:, mt, q0:q0 + 512], ps,
                                             AF.Identity,
                                             bias=b1t[:, mt:mt + 1], scale=1.0 / WS)
                    else:
                        gl = work.tile([128, 512], bf, tag="gel", name="gel")
                        nc.scalar.activation(gl, ps, AF.Gelu,
                                             bias=b1t[:, mt:mt + 1], scale=1.0 / WS)
                        mu = mt - NMT // 2
                        nc.vector.tensor_mul(out=Uff[:, mu, q0:q0 + 512],
                                             in0=Uff[:, mu, q0:q0 + 512], in1=gl)
                for tt in range(4):
                    qs = qh * 4 + tt
                    po = ps_acc([128, DIM], name="ff2_ps")
                    for p in range(IFF // 256):
                        nc.tensor.matmul(po,
                                         lhsT=Uff[:, 2 * p:2 * p + 2,
                                                  qs * 128:(qs + 1) * 128],
                                         rhs=wsb["ff_W2"][:, 2 * p:2 * p + 2, :],
                                         start=(p == 0), stop=(p == IFF // 256 - 1),
                                         perf_mode=PM.DoubleRow)
                    ot = work.tile([128, DIM], f32, tag="ot", name="ot")
                    nc.vector.scalar_tensor_tensor(
                        out=ot, in0=po, scalar=1.0 / WS, in1=resid[:, qs, :],
                        op0=OP.mult, op1=OP.add)
                    if "ff_b2" in flags:
                        nc.vector.tensor_add(out=ot, in0=ot, in1=bcast["ff_b2"])
                    nc.sync.dma_start(out=out_d[qs * 128:(qs + 1) * 128, :], in_=ot)

    nc.compile()
    return nc


_CACHE = {}


def _get_nc(S, R, flags):
    key = (S, R, tuple(sorted(flags)))
    if key not in _CACHE:
        _CACHE[key] = build_nc(S, R, flags)
    return _CACHE[key]


def _pad_qk8(w):
    """Q/K weight layout for fp8 DoubleRow scores: per head h (g=h//4,
    m=h%4), sub i: out col 128*(2g+i) + 32*m + dk <- w col 40h + 20i + dk."""
    w = np.asarray(w)
    out = np.zeros((w.shape[0], 512), np.float32)
    for h in range(HEADS):
        g, m = divmod(h, 4)
        for i in range(2):
            c0 = 128 * (2 * g + i) + 32 * m
            out[:, c0:c0 + 20] = w[:, DH * h + 20 * i:DH * h + 20 * i + 20]
    return out


def _pad_rows(w, rows):
    w = np.asarray(w)
    out = np.zeros((rows, w.shape[1]), np.float32)
    out[:w.shape[0]] = w
    return out


def make_in_maps(x, context, ln_params, weights):
    """Host-side prep: returns (flags, in_maps, R, S, Bn)."""
    x = np.asarray(x)
    context = np.asarray(context)
    Bn = x.shape[0]
    S = x.shape[1]
    R = S * Bn // NCORES
    flags = set()
    for nm in ("ln1_w", "ln2_w", "ln3_w"):
        if not np.allclose(np.asarray(ln_params[nm]), 1.0):
            flags.add(nm)
    for nm in ("ln1_b", "ln2_b", "ln3_b", "a1_bo", "a2_bo", "ff_b2"):
        if not np.allclose(np.asarray(ln_params[nm]), 0.0):
            flags.add(nm)
    weights = {nm: np.asarray(w).astype(np.float32) * WS
               for nm, w in weights.items()}
    for nm in ("a1_Wq", "a1_Wk", "a2_Wq", "a2_Wk"):
        weights[nm] = _pad_qk8(weights[nm])
    for nm in ("a1_Wq", "a1_Wk", "a1_Wv", "a2_Wq", "ff_W1"):
        weights[nm] = _pad_rows(weights[nm], 512)
    shared = {nm: np.ascontiguousarray(w.astype(E4M3))
              for nm, w in weights.items()}
    shared["ff_b1"] = np.ascontiguousarray(
        np.asarray(ln_params["ff_b1"]).astype(np.float32))
    for nm in flags:
        shared[nm] = np.ascontiguousarray(
            np.asarray(ln_params[nm]).astype(np.float32))
    xf32_ = x.astype(np.float32)
    xmean = xf32_.mean(-1)
    xrstd = 1.0 / np.sqrt(xf32_.var(-1) + EPS)
    xstats = np.ascontiguousarray(
        np.stack([xmean, xrstd], axis=-1).astype(np.float32))
    xbf = np.ascontiguousarray(x.astype(BF16))
    ctxT = np.ascontiguousarray(
        np.asarray(context).astype(E4M3).transpose(0, 2, 1))
    xf32 = np.ascontiguousarray(x.astype(np.float32))
    in_maps = []
    cpb = NCORES // Bn
    for core in range(NCORES):
        b, c = divmod(core, cpb)
        m = dict(shared)
        m["xfull"] = xbf[b]
        m["xq"] = np.ascontiguousarray(xf32[b, c * R:(c + 1) * R])
        m["xqb"] = np.ascontiguousarray(xbf[b, c * R:(c + 1) * R])
        m["xfs"] = xstats[b]
        m["xqs"] = np.ascontiguousarray(xstats[b, c * R:(c + 1) * R])
        m["ctxT"] = ctxT[b]
        in_maps.append(m)
    return flags, in_maps, R, S, Bn


def kernel(x, context, ln1_w, ln1_b, ln2_w, ln2_b, ln3_w, ln3_b,
           a1_Wq, a1_Wk, a1_Wv, a1_Wo, a1_bo,
           a2_Wq, a2_Wk, a2_Wv, a2_Wo, a2_bo,
           ff_W1, ff_b1, ff_W2, ff_b2, _trace=False):
    from concourse.bass_utils import run_bass_kernel_spmd

    weights = dict(a1_Wq=a1_Wq, a1_Wk=a1_Wk, a1_Wv=a1_Wv, a1_Wo=a1_Wo,
                   a2_Wq=a2_Wq, a2_Wk=a2_Wk, a2_Wv=a2_Wv, a2_Wo=a2_Wo,
                   ff_W1=ff_W1, ff_W2=ff_W2)
    ln_params = dict(ln1_w=ln1_w, ln1_b=ln1_b, ln2_w=ln2_w, ln2_b=ln2_b,
                     ln3_w=ln3_w, ln3_b=ln3_b, a1_bo=a1_bo, a2_bo=a2_bo,
                     ff_b1=ff_b1, ff_b2=ff_b2)
    flags, in_maps, R, S, Bn = make_in_maps(x, context, ln_params, weights)
    nc = _get_nc(S, R, flags)
    res = run_bass_kernel_spmd(nc, in_maps, core_ids=list(range(NCORES)),
                               trace=_trace)
    out = np.empty((Bn, S, DIM), np.float32)
    cpb = NCORES // Bn
    for core in range(NCORES):
        b, c = divmod(core, cpb)
        out[b, c * R:(c + 1) * R] = res.results[core]["out"]
    kernel._last_result = res
    return out


# revision 50
# speedup vs baseline: 1.1993x; 1.1993x over previous
"""Trainium2 Bass kernel for a BasicTransformerBlock (self-attn + cross-attn + GEGLU FF).

Sharding: 8 cores = 2 batches x 4 sequence chunks of 1024 rows. Each core
redundantly computes LN1 + K/V projections over its batch's full 4096 rows
(position-independent, so all cores run an identical SPMD program) and
produces its own 1024-row slice of the output. No collectives.

Precision: fp32 residual stream; fp8e4m3 weights/activations with DoubleRow
matmuls for every projection and the attention score / probability-x-V
products. On real TRN2 silicon DoubleRow does NOT speed up the moving-dim
stream (~0.55ns/output-col regardless of dtype at the observed clock) —
its win is doubling contract per pass, which halves the pass count of every
projection (contract 256+ per DR matmul). Weights are pre-scaled x16 on the
host so their 0.02-sigma values sit in fp8's normal range; the 1/16 (or
1/256 for Q.K) is folded into activation scales, the softmax ones-column
(set to 16), and the residual-add epilogues. Softmax runs without
max-subtraction (scores are provably small at this scale) with the scale
folded into exp; the denominator comes free from a ones-column in V.

Engine budget: LN rstd is computed entirely on DVE (batched stats + a
Newton rsqrt from a 1/x seed), so outside the FF gelu the ACT engine runs a
single activation table — no 1.3us ACT_TABLE_LOAD thrash. Softmax exp is
split 5:3 between ACT (table exp) and DVE ((1+z/32)^32 custom op); all
PSUM->SBUF copies round-robin ACT/DVE (GpSimd cannot read PSUM and its
tensor ops are ~6x slower than DVE, so it only does memsets). Known floor:
the score stream (262k PE columns) and the LDWEIGHTS-bound PV phase
(~0.47ns per stationary column, reloaded per 128-q-block) dominate; per-core
clock state varies run-to-run by ~20% (normalize by the EXP-op average when
comparing traces).
"""

import numpy as np
import ml_dtypes

DIM = 320
HEADS = 8
DH = 40
CTX = 768
IFF = 1280  # GEGLU inner width; proj1 width = 2*IFF
EPS = 1e-5
SCALE = DH ** -0.5
NCORES = 8
MCTX = 77
VS = 336  # V row stride (8*41 = 328 padded to %16 for DoubleRow pair stride)
WS = 16.0  # host-side weight scale (folded back out on-device)

BF16 = ml_dtypes.bfloat16
E4M3 = ml_dtypes.float8_e4m3


def _register_exp_op():
    """Custom DVE op: out = (in0*s0 + s1)^32 — used as exp(z) ~ (1+z/32)^32
    to offload part of the softmax exp from ACT to the vector engine."""
    import concourse.dve_ops as dve_ops
    for o in dve_ops.OPS:
        if o.name == "EXP_POLY32_ANT":
            return o
    from concourse.dve_spec import Spec, Src0, C0, C1, sq
    spec = Spec(
        body=sq(sq(sq(sq(sq(Src0 * C0 + C1))))),
        reference=lambda in0, in1, s0, s1, imm2:
            ((in0.astype(np.float32) * s0 + s1) ** 32).astype(np.float32))
    op = dve_ops.DveOp("EXP_POLY32_ANT", spec, subdim=False,
                       uops_sha={"v3": "eafb894a1d5c531b"})
    dve_ops.OPS.append(op)
    dve_ops._SUB_OPCODE_FOR_NAME[op.name] = \
        dve_ops._CUSTOM_DVE_ROW_BASE + len(dve_ops.OPS) - 1
    dve_ops.CUSTOM_DVE_SPECS[op.name] = op.spec
    return op


def build_nc(S, R, flags=()):
    """Build + compile the per-core Bass program.

    flags: subset of {"ln1_w","ln1_b","ln2_w","ln2_b","ln3_w","ln3_b",
    "a1_bo","a2_bo","ff_b2"} that are non-trivial and must be applied.
    """
    import concourse.bass as bass
    import concourse.tile as tile
    from concourse import bacc, mybir
    from concourse.masks import make_identity

    f32 = mybir.dt.float32
    bf = mybir.dt.bfloat16
    f8 = mybir.dt.float8e4
    AF = mybir.ActivationFunctionType
    OP = mybir.AluOpType
    PM = mybir.MatmulPerfMode
    flags = set(flags)

    KB = S // 128     # key blocks (self-attn)
    KBP = KB // 2     # key block pairs (DoubleRow PV)
    QT = R // 128     # q row-tiles
    QHS = R // 512    # q 512-row groups
    ISC = SCALE / (WS * WS)  # exp scale with both Q,K weight scales folded

    nc = bacc.Bacc("TRN2", target_bir_lowering=False, debug=False)

    def din(name, shape, dt=f8):
        return nc.dram_tensor(name, shape, dt, kind="ExternalInput").ap()

    xfull_d = din("xfull", [S, DIM], bf)
    xq_d = din("xq", [R, DIM], f32)
    xqb_d = din("xqb", [R, DIM], bf)
    xqs_d = din("xqs", [R, 2], f32)
    xfs_d = din("xfs", [S, 2], f32)
    ctxT_d = din("ctxT", [CTX, MCTX])
    # weights: [in(padded), out] fp8e4m3, host-prescaled x16
    w_d = {}
    for nm, shape in [
        ("a1_Wq", [512, 512]), ("a1_Wk", [512, 512]), ("a1_Wv", [512, DIM]),
        ("a1_Wo", [DIM, DIM]), ("a2_Wq", [512, 512]), ("a2_Wk", [CTX, 512]),
        ("a2_Wv", [CTX, DIM]), ("a2_Wo", [DIM, DIM]),
        ("ff_W1", [512, 2 * IFF]), ("ff_W2", [IFF, DIM]),
    ]:
        w_d[nm] = din(nm, shape)
    b1_d = din("ff_b1", [2 * IFF], f32)
    vec_d = {nm: din(nm, [DIM], f32) for nm in sorted(flags)}
    out_d = nc.dram_tensor("out", [R, DIM], f32, kind="ExternalOutput").ap()

    NCH = {512: 4, CTX: 6, IFF: 10, DIM: 3}  # input-dim -> #128-chunks

    with tile.TileContext(nc) as tc:
        import contextlib
        with contextlib.ExitStack() as est:
            persist = est.enter_context(tc.tile_pool(name="persist", bufs=1))
            work = est.enter_context(tc.tile_pool(name="work", bufs=4))
            expp = est.enter_context(tc.tile_pool(name="expp", bufs=6))
            # One PSUM pool: tag "sc" = 2 x [128,1024]f32 (4 banks), tag
            # "acc" = 4 x [128,512]f32 (4 banks). All other PSUM tiles
            # allocate from these tags so phases can pipeline.
            psum = est.enter_context(tc.tile_pool(name="psum", bufs=2,
                                                  space="PSUM"))

            def ps_sc(shape, dt=f32, name="sc"):
                return psum.tile(shape, dt, tag="sc", bufs=2, name=name)

            def ps_acc(shape, dt=f32, name="accp"):
                return psum.tile(shape, dt, tag="acc", bufs=4, name=name)

            ident = persist.tile([128, 128], bf, name="ident")
            make_identity(nc, ident)
            # ---- persistent activations (feature-major, 4 chunk-slots for
            # DoubleRow pairing; chunk2[64:] and chunk3 zeroed once)
            h1T = persist.tile([128, 4, S], f8, name="h1T")
            actT = persist.tile([128, 4, R], f8, name="actT")
            Kf = persist.tile([128, 4, S], f8, name="Kf")
            Qf = persist.tile([128, 4, R], f8, name="Qf")
            Vr = persist.tile([128, KB, VS], f8, name="Vr")
            K2f = persist.tile([128, 4, 80], f8, name="K2f")
            Q2f = persist.tile([128, 4, R], f8, name="Q2f")
            V2r = persist.tile([128, VS], f8, name="V2r")
            resid = persist.tile([128, QT, DIM], f32, name="resid")
            Uff = persist.tile([128, IFF // 128, R], f8, name="Uff")

            # actT pads on the (startup-idle) DVE so they don't queue
            # behind make_identity on GpSimd; h1T pads on GpSimd (only
            # needed once K-proj starts)
            nc.vector.memset(actT[64:128, 2, :], 0.0)
            nc.vector.memset(actT[:, 3, :], 0.0)
            nc.gpsimd.memset(h1T[64:128, 2, :], 0.0)
            nc.gpsimd.memset(h1T[:, 3, :], 0.0)

            # ---- weights into SBUF, [in, out] layout chunked on partitions.
            wsb = {}

            def load_w(names):
                for nm in names:
                    ind, width = w_d[nm].shape
                    nch = NCH[ind]
                    t = persist.tile([128, nch, width], f8, name=f"w_{nm}",
                                     uniquify=True)
                    for c in range(nch):
                        kw = min(128, ind - c * 128)
                        nc.sync.dma_start(out=t[:kw, c, :],
                                          in_=w_d[nm][c * 128:c * 128 + kw, :])
                    wsb[nm] = t

            xqs = persist.tile([128, QT, 2], f32, name="xqs")
            xfs = persist.tile([128, KB, 2], f32, name="xfs")
            nc.sync.dma_start(out=xqs,
                              in_=xqs_d.rearrange("(t p) s -> p t s", p=128))
            nc.sync.dma_start(out=xfs,
                              in_=xfs_d.rearrange("(t p) s -> p t s", p=128))
            xqb = persist.tile([128, QT, DIM], bf, name="xqb")
            for t in range(QT):
                nc.sync.dma_start(out=xqb[:, t, :],
                                  in_=xqb_d[t * 128:(t + 1) * 128, :])
            load_w(["a1_Wq", "a1_Wk", "a1_Wv"])
            for t in range(QT):
                nc.sync.dma_start(out=resid[:, t, :],
                                  in_=xq_d[t * 128:(t + 1) * 128, :])

            bcast = {}
            for nm in sorted(flags):
                t = persist.tile([128, DIM], f32, name=f"bc_{nm}")
                src = vec_d[nm]
                bc_ap = bass.AP(tensor=src.tensor, offset=src.offset,
                                ap=[[0, 128]] + [list(p) for p in src.ap])
                nc.gpsimd.dma_start(out=t, in_=bc_ap)
                bcast[nm] = t

            # round-robin engine pickers for PSUM->SBUF copies
            _cp = [0]

            def copy_eng():
                _cp[0] += 1
                return "act" if _cp[0] % 2 == 0 else "dve"

            def ps_copy(out, in_, eng=None, scale=None, bias=None):
                eng = eng or copy_eng()
                if eng == "act":
                    nc.scalar.activation(out, in_, AF.Identity,
                                         bias=bias if bias is not None else 0.0,
                                         scale=scale if scale is not None else 1.0)
                else:
                    assert bias is None and scale is None
                    nc.vector.tensor_copy(out=out, in_=in_)

            def ln_stats_alloc(n):
                return work.tile([128, n, 2], f32, tag="mv", bufs=3,
                                 name="mvall")

            def ln_stat1(mvall, t, src_ap):
                stats = work.tile([128, 6], f32, tag="bnst", name="stats")
                nc.vector.bn_stats(stats, src_ap)
                nc.vector.bn_aggr(mvall[:, t, :], stats)

            def ln_finish(mvall, n, src_fn, wkey, bkey, consume_fn):
                """rstd = rsqrt(var) via seed y0=(1+1/v)/2 + 3 Newton steps
                (DVE only — no ACT table functions), then per-tile normalize
                handed to consume_fn(t, h)."""
                var_ap = mvall[:, :, 1]
                rstd = work.tile([128, n], f32, tag="rstd", name="rstd")
                tmp = work.tile([128, n], f32, tag="rtmp", name="rtmp")
                nc.vector.reciprocal(rstd, var_ap)
                nc.vector.tensor_scalar(out=rstd, in0=rstd, scalar1=0.5,
                                        scalar2=0.5, op0=OP.mult, op1=OP.add)
                for _ in range(2):
                    nc.vector.tensor_mul(out=tmp, in0=rstd, in1=rstd)
                    nc.vector.tensor_mul(out=tmp, in0=tmp, in1=var_ap)
                    nc.vector.tensor_scalar(out=tmp, in0=tmp, scalar1=-0.5,
                                            scalar2=1.5, op0=OP.mult, op1=OP.add)
                    nc.vector.tensor_mul(out=rstd, in0=rstd, in1=tmp)
                for t in range(n):
                    h = work.tile([128, DIM], bf, tag="h", bufs=6, name="h")
                    nc.vector.tensor_scalar(
                        out=h, in0=src_fn(t), scalar1=mvall[:, t, 0:1],
                        scalar2=rstd[:, t:t + 1], op0=OP.subtract, op1=OP.mult)
                    if wkey in flags:
                        nc.vector.tensor_mul(out=h, in0=h, in1=bcast[wkey])
                    if bkey in flags:
                        nc.vector.tensor_add(out=h, in0=h, in1=bcast[bkey])
                    consume_fn(t, h)

            def ln_single(src_ap, wkey, bkey, consume_fn):
                """Latency-optimized per-tile LN: full stats->rsqrt->normalize
                chain for ONE 128-row tile, so its transpose can issue without
                waiting for a whole phase's stats batch."""
                mv = ln_stats_alloc(1)
                ln_stat1(mv, 0, src_ap)
                ln_finish(mv, 1, lambda _t: src_ap, wkey, bkey,
                          lambda _t, h: consume_fn(h))

            def ln_norm_host(src_ap, st, t, wkey, bkey):
                """LN1 normalize using host-precomputed (mean, rstd)."""
                h = work.tile([128, DIM], bf, tag="h", bufs=6, name="h")
                nc.vector.tensor_scalar(
                    out=h, in0=src_ap, scalar1=st[:, t, 0:1],
                    scalar2=st[:, t, 1:2], op0=OP.subtract, op1=OP.mult)
                if wkey in flags:
                    nc.vector.tensor_mul(out=h, in0=h, in1=bcast[wkey])
                if bkey in flags:
                    nc.vector.tensor_add(out=h, in0=h, in1=bcast[bkey])
                return h

            def ln_batch(n, src_fn, wkey, bkey, consume_fn):
                mvall = ln_stats_alloc(n)
                for t in range(n):
                    ln_stat1(mvall, t, src_fn(t))
                ln_finish(mvall, n, src_fn, wkey, bkey, consume_fn)

            def transpose_into(dstT, src_bf, col0, ps_fn=None):
                for c in range(3):
                    kw = 128 if c < 2 else 64
                    pt = (ps_fn or ps_sc)([128, 128], bf, name="tr_ps")
                    nc.tensor.transpose(pt[:kw, :], src_bf[:, c * 128:c * 128 + kw],
                                        ident)
                    ps_copy(dstT[:kw, c, col0:col0 + 128], pt[:kw, :])

            def proj_fm(dst, wt, srcT, n_lo, n_hi, npair, eng=None):
                """Feature-major projection, DoubleRow over chunk pairs.
                dst[:, g, n0:n0+nw] = (wt[:,:,128g:128(g+1)]).T @ srcT[:,:,n]"""
                ng = wt.shape[2] // 128
                for g in range(ng):
                    for n0 in range(n_lo, n_hi, 512):
                        nw = min(512, n_hi - n0)
                        ps = ps_acc([128, 512], name="proj_ps")
                        for p in range(npair):
                            nc.tensor.matmul(
                                ps[:, :nw],
                                lhsT=wt[:, 2 * p:2 * p + 2, 128 * g:128 * g + 128],
                                rhs=srcT[:, 2 * p:2 * p + 2, n0:n0 + nw],
                                start=(p == 0), stop=(p == npair - 1),
                                perf_mode=PM.DoubleRow)
                        ps_copy(dst[:, g, n0:n0 + nw], ps[:, :nw], eng=eng)

            exp_op = _register_exp_op()

            def load_late_weights():
                load_w(["a1_Wo", "a2_Wq", "a2_Wk", "a2_Wv", "a2_Wo", "ff_W1",
                        "ff_W2"])
                b1 = persist.tile([128, (2 * IFF) // 128], f32, name="b1t")
                nc.sync.dma_start(out=b1, in_=b1_d.rearrange("(c p) -> p c", p=128))
                ctxm = persist.tile([128, 6, 80], f8, name="ctxT_sb")
                nc.gpsimd.memset(ctxm, 0.0)
                for c in range(6):
                    nc.sync.dma_start(out=ctxm[:, c, :MCTX],
                                      in_=ctxT_d[c * 128:(c + 1) * 128, :])
                return b1, ctxm

            def cross_kv():
                # K2: [128,4,80] fp8 in qk8 layout; V2: [77, 328] + ones(16)
                for g in range(4):
                    ps = ps_sc([128, 128], name="k2_ps")
                    for p in range(3):
                        nc.tensor.matmul(
                            ps[:, :80],
                            lhsT=wsb["a2_Wk"][:, 2 * p:2 * p + 2,
                                              128 * g:128 * g + 128],
                            rhs=ctxT_sb[:, 2 * p:2 * p + 2, :],
                            start=(p == 0), stop=(p == 2),
                            perf_mode=PM.DoubleRow)
                    nc.vector.tensor_copy(out=K2f[:, g, :], in_=ps[:, :80])
                ps = ps_acc([128, 512], name="v2_ps")
                for p in range(3):
                    nc.tensor.matmul(
                        ps[:80, :DIM],
                        lhsT=ctxT_sb[:, 2 * p:2 * p + 2, :],
                        rhs=wsb["a2_Wv"][:, 2 * p:2 * p + 2, :],
                        start=(p == 0), stop=(p == 2),
                        perf_mode=PM.DoubleRow)
                nc.vector.tensor_copy(
                    out=V2r[:MCTX, 0:328].rearrange("p (h c) -> p h c", c=41)[:, :, 0:40],
                    in_=ps[:MCTX, :DIM].rearrange("p (h c) -> p h c", c=40))
                nc.vector.memset(
                    V2r[:MCTX, 0:328].rearrange("p (h c) -> p h c",
                                                c=41)[:, :, 40:41], WS)

            xf = persist.tile([128, KB, DIM], bf, name="xf")
            for t in range(KB):
                nc.sync.dma_start(out=xf[:, t, :],
                                  in_=xfull_d[t * 128:(t + 1) * 128, :])

            # ---- own rows: LN1 -> actT, Qf (stats host-precomputed)
            for t in range(QT):
                h = ln_norm_host(xqb[:, t, :], xqs, t, "ln1_w", "ln1_b")
                transpose_into(actT, h, t * 128)
            proj_fm(Qf, wsb["a1_Wq"], actT, 0, R, 2, eng="act")

            # ---- attn1 building blocks
            def qk8_lhs_rhs(h, kb, q0):
                g, m = divmod(h, 4)
                lhsT = Kf[32 * m:32 * m + 20, 2 * g:2 * g + 2,
                          kb * 128:(kb + 1) * 128]
                rhs = Qf[32 * m:32 * m + 20, 2 * g:2 * g + 2, q0:q0 + 512]
                return lhsT, rhs, (32 * m, 0)

            def attn1_scores_exp(q0, hp, kbp, ept):
                """Two kb blocks of scores+exp for heads (2hp, 2hp+1) into
                ept[:, i, :]. Exp split between ACT and DVE."""
                for i in range(2):
                    kb = 2 * kbp + i
                    sc = ps_sc([128, 1024], name="sc")
                    for j in range(2):
                        h = 2 * hp + j
                        lhsT, rhs, tp = qk8_lhs_rhs(h, kb, q0)
                        nc.tensor.matmul(sc[:, j * 512:(j + 1) * 512],
                                         lhsT=lhsT, rhs=rhs, start=True,
                                         stop=True, perf_mode=PM.DoubleRow,
                                         tile_position=tp)
                    if (2 * kbp + i + 3 * hp) % 8 < 3:
                        # exp(z) ~ (1+z/32)^32 on the vector engine
                        nc.vector._custom_dve(exp_op, out=ept[:, i, :], in0=sc,
                                              s0=ISC / 32.0, s1=1.0)
                    else:
                        nc.scalar.activation(ept[:, i, :], sc, AF.Exp, scale=ISC)

            def attn1_pv(acc, hp, kbp, ept):
                for j in range(2):
                    h = 2 * hp + j
                    for qs in range(4):
                        nc.tensor.matmul(
                            acc[qs][:, 41 * h:41 * h + 41],
                            lhsT=ept[:, :, j * 512 + qs * 128:
                                     j * 512 + (qs + 1) * 128],
                            rhs=Vr[:, 2 * kbp:2 * kbp + 2, 41 * h:41 * h + 41],
                            start=(kbp == 0), stop=(kbp == KBP - 1),
                            perf_mode=PM.DoubleRow, skip_group_check=True)

            # ---- LN1 + K/V production per 512-column block
            def kv_block(nb):
                for g in range(4):
                    ps = ps_acc([128, 512], name="kf_ps")
                    for p in range(2):
                        nc.tensor.matmul(
                            ps,
                            lhsT=wsb["a1_Wk"][:, 2 * p:2 * p + 2,
                                              128 * g:128 * g + 128],
                            rhs=h1T[:, 2 * p:2 * p + 2, nb * 512:(nb + 1) * 512],
                            start=(p == 0), stop=(p == 1),
                            perf_mode=PM.DoubleRow)
                    ps_copy(Kf[:, g, nb * 512:(nb + 1) * 512], ps)
                for tt in range(4):
                    t = nb * 4 + tt
                    ps = ps_acc([128, 512], name="v_ps")  # noqa: PLW2901
                    for p in range(2):
                        nc.tensor.matmul(
                            ps[:, :DIM],
                            lhsT=h1T[:, 2 * p:2 * p + 2, t * 128:(t + 1) * 128],
                            rhs=wsb["a1_Wv"][:, 2 * p:2 * p + 2, :],
                            start=(p == 0), stop=(p == 1),
                            perf_mode=PM.DoubleRow)
                    nc.vector.tensor_copy(
                        out=Vr[:, t, 0:328].rearrange("p (h c) -> p h c",
                                                      c=41)[:, :, 0:40],
                        in_=ps[:, :DIM].rearrange("p (h c) -> p h c", c=40))
                    if t % 8 == 7 or t == KB - 1:
                        lo = t - (t % 8)
                        nc.vector.memset(
                            Vr[:, lo:t + 1, 0:328].rearrange(
                                "p b (h c) -> p b h c", c=41)[:, :, :, 40], WS)

            def kv_consume(t, h):
                transpose_into(h1T, h, t * 128)
                if t % 4 == 3:
                    kv_block(t // 4)

            for t in range(KB):
                h = ln_norm_host(xf[:, t, :], xfs, t, "ln1_w", "ln1_b")
                kv_consume(t, h)

            # ---- shared attention epilogue: normalize, transpose, proj, add
            def finish_attn(qh, acc, wo, bo_key, after_qs=None):
                for qs in range(4):
                    rec = work.tile([128, HEADS], f32, tag="rec", name="rec")
                    nc.vector.reciprocal(
                        rec, acc[qs].rearrange("p (h c) -> p h c", c=41)[:, :, 40])
                    arm = work.tile([128, DIM], bf, tag="arm", name="arm")
                    rb = bass.AP(tensor=rec.tensor, offset=rec.offset,
                                 ap=[list(rec.ap[0]), [rec.ap[1][0], HEADS],
                                     [0, 40]])
                    nc.vector.tensor_mul(
                        out=arm.rearrange("p (h c) -> p h c", c=40),
                        in0=acc[qs].rearrange("p (h c) -> p h c", c=41)[:, :, 0:40],
                        in1=rb)
                    afm = work.tile([128, 3, 128], f8, tag="afm", name="afm")
                    for c in range(3):
                        kw = 128 if c < 2 else 64
                        pt = ps_acc([128, 128], bf, name="afm_tr")
                        nc.tensor.transpose(pt[:kw, :],
                                            arm[:, c * 128:c * 128 + kw], ident)
                        ps_copy(afm[:kw, c, :], pt[:kw, :])
                    po = ps_acc([128, DIM], name="po")
                    nc.tensor.matmul(po, lhsT=afm[:, 0:2, :], rhs=wo[:, 0:2, :],
                                     start=True, stop=False,
                                     perf_mode=PM.DoubleRow)
                    nc.tensor.matmul(po, lhsT=afm[:64, 2, :], rhs=wo[:64, 2, :],
                                     start=False, stop=True)
                    t = qh * 4 + qs
                    # resid += po/WS  (weight prescale folded out)
                    nc.vector.scalar_tensor_tensor(
                        out=resid[:, t, :], in0=po, scalar=1.0 / WS,
                        in1=resid[:, t, :], op0=OP.mult, op1=OP.add)
                    if bo_key in flags:
                        nc.vector.tensor_add(out=resid[:, t, :],
                                             in0=resid[:, t, :],
                                             in1=bcast[bo_key])
                    if after_qs is not None:
                        after_qs(qs)

            NMT = (2 * IFF) // 128  # 20
            # ================= per q-half: attn1 -> attn2 -> FF (pipelined)
            for qh in range(QHS):
                q0 = qh * 512
                # ---- self-attention (PV software-pipelined one pair back)
                acc = [ps_acc([128, HEADS * 41], name=f"acc{qs}")
                       for qs in range(4)]
                pending = []
                for hp in range(HEADS // 2):
                    for kbp in range(KBP):
                        ept = expp.tile([128, 2, 1024], f8, tag="ep", name="ep")
                        attn1_scores_exp(q0, hp, kbp, ept)
                        pending.append((hp, kbp, ept))
                        while len(pending) > 2:
                            attn1_pv(acc, *pending.pop(0))
                for phk in pending:
                    attn1_pv(acc, *phk)
                if qh == 0:
                    # cross-attn/FF weights + context K,V: DMA'd and computed
                    # here so they hide under attn1(qh0) instead of stalling
                    # the PE before it starts
                    b1t, ctxT_sb = load_late_weights()
                    cross_kv()
                finish_attn(qh, acc, wsb["a1_Wo"], "a1_bo",
                            after_qs=lambda qs: ln_single(
                                resid[:, qh * 4 + qs, :], "ln2_w", "ln2_b",
                                lambda h, _q=qs: transpose_into(
                                    actT, h, (qh * 4 + _q) * 128)))

                # ---- cross-attention for this q-half
                proj_fm(Q2f, wsb["a2_Wq"], actT, q0, q0 + 512, 2)
                acc = [ps_acc([128, HEADS * 41], name=f"acc2_{qs}")
                       for qs in range(4)]
                p2 = []
                for hp in range(HEADS // 2):
                    sc = ps_sc([128, 1024], name="sc2")
                    for j in range(2):
                        h = 2 * hp + j
                        g, m = h // 4, h % 4
                        nc.tensor.matmul(
                            sc[:80, j * 512:(j + 1) * 512],
                            lhsT=K2f[32 * m:32 * m + 20, 2 * g:2 * g + 2, :],
                            rhs=Q2f[32 * m:32 * m + 20, 2 * g:2 * g + 2,
                                    q0:q0 + 512],
                            start=True, stop=True, perf_mode=PM.DoubleRow,
                            tile_position=(32 * m, 0))
                    ep = expp.tile([128, 1024], f8, tag="ep2", bufs=4, name="ep2")
                    if hp % 2 == 1:
                        nc.vector._custom_dve(exp_op, out=ep[:MCTX, :],
                                              in0=sc[:MCTX, :],
                                              s0=ISC / 32.0, s1=1.0)
                    else:
                        nc.scalar.activation(ep[:MCTX, :], sc[:MCTX, :], AF.Exp,
                                             scale=ISC)
                    p2.append((hp, ep))
                for hp, ep in p2:
                    for j in range(2):
                        h = 2 * hp + j
                        for qs in range(4):
                            nc.tensor.matmul(
                                acc[qs][:, 41 * h:41 * h + 41],
                                lhsT=ep[:MCTX, j * 512 + qs * 128:
                                        j * 512 + (qs + 1) * 128],
                                rhs=V2r[:MCTX, 41 * h:41 * h + 41],
                                start=True, stop=True, skip_group_check=True)
                finish_attn(qh, acc, wsb["a2_Wo"], "a2_bo",
                            after_qs=lambda qs: ln_single(
                                resid[:, qh * 4 + qs, :], "ln3_w", "ln3_b",
                                lambda h, _q=qs: transpose_into(
                                    actT, h, (qh * 4 + _q) * 128)))

                # ---- GEGLU FF for this q-half
                _order = [m for pair in zip(range(NMT // 2), range(NMT // 2, NMT))
                          for m in pair]
                for mt in _order:
                    ps = ps_acc([128, 512], name="ff1_ps")
                    for p in range(2):
                        nc.tensor.matmul(
                            ps,
                            lhsT=wsb["ff_W1"][:, 2 * p:2 * p + 2,
                                              mt * 128:(mt + 1) * 128],
                            rhs=actT[:, 2 * p:2 * p + 2, q0:q0 + 512],
                            start=(p == 0), stop=(p == 1),
                            perf_mode=PM.DoubleRow)
                    if mt < NMT // 2:
                        nc.scalar.activation(Uff[:, mt, q0:q0 + 512], ps,
                                             AF.Identity,
                                             bias=b1t[:, mt:mt + 1], scale=1.0 / WS)
                    else:
                        gl = work.tile([128, 512], bf, tag="gel", name="gel")
                        nc.scalar.activation(gl, ps, AF.Gelu,
                                             bias=b1t[:, mt:mt + 1], scale=1.0 / WS)
                        mu = mt - NMT // 2
                        nc.vector.tensor_mul(out=Uff[:, mu, q0:q0 + 512],
                                             in0=Uff[:, mu, q0:q0 + 512], in1=gl)
                for tt in range(4):
                    qs = qh * 4 + tt
                    po = ps_acc([128, DIM], name="ff2_ps")
                    for p in range(IFF // 256):
                        nc.tensor.matmul(po,
                                         lhsT=Uff[:, 2 * p:2 * p + 2,
                                                  qs * 128:(qs + 1) * 128],
                                         rhs=wsb["ff_W2"][:, 2 * p:2 * p + 2, :],
                                         start=(p == 0), stop=(p == IFF // 256 - 1),
                                         perf_mode=PM.DoubleRow)
                    ot = work.tile([128, DIM], f32, tag="ot", name="ot")
                    nc.vector.scalar_tensor_tensor(
                        out=ot, in0=po, scalar=1.0 / WS, in1=resid[:, qs, :],
                        op0=OP.mult, op1=OP.add)
                    if "ff_b2" in flags:
                        nc.vector.tensor_add(out=ot, in0=ot, in1=bcast["ff_b2"])
                    nc.sync.dma_start(out=out_d[qs * 128:(qs + 1) * 128, :], in_=ot)

    nc.compile()
    return nc


_CACHE = {}


def _get_nc(S, R, flags):
    key = (S, R, tuple(sorted(flags)))
    if key not in _CACHE:
        _CACHE[key] = build_nc(S, R, flags)
    return _CACHE[key]


def _pad_qk8(w):
    """Q/K weight layout for fp8 DoubleRow scores: per head h (g=h//4,
    m=h%4), sub i: out col 128*(2g+i) + 32*m + dk <- w col 40h + 20i + dk."""
    w = np.asarray(w)
    out = np.zeros((w.shape[0], 512), np.float32)
    for h in range(HEADS):
        g, m = divmod(h, 4)
        for i in range(2):
            c0 = 128 * (2 * g + i) + 32 * m
            out[:, c0:c0 + 20] = w[:, DH * h + 20 * i:DH * h + 20 * i + 20]
    return out


def _pad_rows(w, rows):
    w = np.asarray(w)
    out = np.zeros((rows, w.shape[1]), np.float32)
    out[:w.shape[0]] = w
    return out


def make_in_maps(x, context, ln_params, weights):
    """Host-side prep: returns (flags, in_maps, R, S, Bn)."""
    x = np.asarray(x)
    context = np.asarray(context)
    Bn = x.shape[0]
    S = x.shape[1]
    R = S * Bn // NCORES
    flags = set()
    for nm in ("ln1_w", "ln2_w", "ln3_w"):
        if not np.allclose(np.asarray(ln_params[nm]), 1.0):
            flags.add(nm)
    for nm in ("ln1_b", "ln2_b", "ln3_b", "a1_bo", "a2_bo", "ff_b2"):
        if not np.allclose(np.asarray(ln_params[nm]), 0.0):
            flags.add(nm)
    weights = {nm: np.asarray(w).astype(np.float32) * WS
               for nm, w in weights.items()}
    for nm in ("a1_Wq", "a1_Wk", "a2_Wq", "a2_Wk"):
        weights[nm] = _pad_qk8(weights[nm])
    for nm in ("a1_Wq", "a1_Wk", "a1_Wv", "a2_Wq", "ff_W1"):
        weights[nm] = _pad_rows(weights[nm], 512)
    shared = {nm: np.ascontiguousarray(w.astype(E4M3))
              for nm, w in weights.items()}
    shared["ff_b1"] = np.ascontiguousarray(
        np.asarray(ln_params["ff_b1"]).astype(np.float32))
    for nm in flags:
        shared[nm] = np.ascontiguousarray(
            np.asarray(ln_params[nm]).astype(np.float32))
    xf32_ = x.astype(np.float32)
    xmean = xf32_.mean(-1)
    xrstd = 1.0 / np.sqrt(xf32_.var(-1) + EPS)
    xstats = np.ascontiguousarray(
        np.stack([xmean, xrstd], axis=-1).astype(np.float32))
    xbf = np.ascontiguousarray(x.astype(BF16))
    ctxT = np.ascontiguousarray(
        np.asarray(context).astype(E4M3).transpose(0, 2, 1))
    xf32 = np.ascontiguousarray(x.astype(np.float32))
    in_maps = []
    cpb = NCORES // Bn
    for core in range(NCORES):
        b, c = divmod(core, cpb)
        m = dict(shared)
        m["xfull"] = xbf[b]
        m["xq"] = np.ascontiguousarray(xf32[b, c * R:(c + 1) * R])
        m["xqb"] = np.ascontiguousarray(xbf[b, c * R:(c + 1) * R])
        m["xfs"] = xstats[b]
        m["xqs"] = np.ascontiguousarray(xstats[b, c * R:(c + 1) * R])
        m["ctxT"] = ctxT[b]
        in_maps.append(m)
    return flags, in_maps, R, S, Bn


def kernel(x, context, ln1_w, ln1_b, ln2_w, ln2_b, ln3_w, ln3_b,
           a1_Wq, a1_Wk, a1_Wv, a1_Wo, a1_bo,
           a2_Wq, a2_Wk, a2_Wv, a2_Wo, a2_bo,
           ff_W1, ff_b1, ff_W2, ff_b2, _trace=False):
    from concourse.bass_utils import run_bass_kernel_spmd

    weights = dict(a1_Wq=a1_Wq, a1_Wk=a1_Wk, a1_Wv=a1_Wv, a1_Wo=a1_Wo,
                   a2_Wq=a2_Wq, a2_Wk=a2_Wk, a2_Wv=a2_Wv, a2_Wo=a2_Wo,
                   ff_W1=ff_W1, ff_W2=ff_W2)
    ln_params = dict(ln1_w=ln1_w, ln1_b=ln1_b, ln2_w=ln2_w, ln2_b=ln2_b,
                     ln3_w=ln3_w, ln3_b=ln3_b, a1_bo=a1_bo, a2_bo=a2_bo,
                     ff_b1=ff_b1, ff_b2=ff_b2)
    flags, in_maps, R, S, Bn = make_in_maps(x, context, ln_params, weights)
    nc = _get_nc(S, R, flags)
    res = run_bass_kernel_spmd(nc, in_maps, core_ids=list(range(NCORES)),
                               trace=_trace)
    out = np.empty((Bn, S, DIM), np.float32)
    cpb = NCORES // Bn
    for core in range(NCORES):
        b, c = divmod(core, cpb)
        out[b, c * R:(c + 1) * R] = res.results[core]["out"]
    kernel._last_result = res
    return out
